# revision 64
# baseline (speedup 1.0000x reference)
"""MiniMax Lightning Attention on 8 Trainium2 NeuronCores.

Sharding: sequence-parallel. Core c handles batch c//4, token chunk
(c%4)*1024..+1024. The decay recurrence crosses chunk boundaries; each
core computes its local per-chunk decay-weighted KV summary E, an
AllGather shares the 8 summaries, and each core reconstructs its
chunk-start state as a decay-weighted sum.

The global decay attention is block-decomposed at B=128 (exact for any
block size): intra-block masked attention + inter-block recurrent state.
B=128 halves the intra-attention PE columns vs B=256 and aligns blocks
with 128-partition tiles.

Fused single-residency design: all big activations (x, V, q, ys, gate)
stay SBUF-resident in bf16; weights stream in bf16. Heads are processed
in pairs (dim groups of 128). Pass 1 runs a lag-1 software pipeline:
group g's projections issue before group g-1's intra-attention so the
PE never waits on scalar/vector latency. Head-pair matmuls (64-row or
64-col) are emitted adjacently so they run concurrently on distinct PE
row/col groups.
"""

import numpy as np
import ml_dtypes

from contextlib import ExitStack

import concourse.bacc as bacc
import concourse.mybir as mybir
import concourse.tile as tile
from concourse.bass_utils import run_bass_kernel_spmd
from concourse.masks import make_identity


AF = mybir.ActivationFunctionType
ALU = mybir.AluOpType
F32 = mybir.dt.float32
BF16 = mybir.dt.bfloat16

H = 32
D = 64
BS = 128              # attention block size (exact for any B)
HID = 2048
B = 2
S = 4096
NC = 8
T = S // 4            # tokens per core (1024)
NCH = T // 128        # 8 token chunks of 128 == blocks per core
NBLK = T // BS        # 8 blocks per core
G = H // 2            # 16 head pairs (dim groups of 128)
GSPLIT = 8            # collective split point (groups 0:8 / 8:16)
KC = HID // 128       # 16 contraction chunks
LAYER_IDX = 0
NUM_LAYERS = 32
EPS = 1e-5

BF = ml_dtypes.bfloat16
USE_ACT_QUEUE = True
USE_PNS = True      # rmsnorm partition-reduce via tiny matmuls (vs DRAM roundtrip)
USE_DMA_TR = False  # HWDGE xbar transpose measured much slower than PE


def _decay():
    base = 1.0 / 2.0 ** (8.0 / H)
    rate = base ** (np.arange(H, dtype=np.float64) + 1.0)
    factor = 1.0 - LAYER_IDX / (NUM_LAYERS - 1 + 1e-5) + 1e-5
    slope = rate * factor                                  # (H,)
    r = np.arange(BS, dtype=np.float64) + 1.0
    qd = np.exp(-slope[:, None] * r[None, :])              # (H, BS) query decay
    kd = np.exp(-slope[:, None] * (BS - r[None, :]))       # (H, BS) key decay
    ij = r[:, None] - r[None, :]                           # i - j
    dd = np.where(
        ij[None] >= 0, np.exp(-slope[:, None, None] * ij[None]), 0.0
    )                                                      # (H, BS_i, BS_j)
    bd = np.exp(-slope * BS)                               # (H,) block decay
    return slope, qd, kd, dd, bd


def _build_nc():
    nc = bacc.Bacc(num_devices=NC)
    hsT = nc.declare_dram_parameter("hsT", [HID, T], BF16, isOutput=False)
    wvT = nc.declare_dram_parameter("wvT", [HID, H * D], BF16, isOutput=False)
    wqkT = nc.declare_dram_parameter("wqkT", [HID, G, 256], BF16, isOutput=False)
    gwT = nc.declare_dram_parameter("gwT", [HID, HID], BF16, isOutput=False)
    owT = nc.declare_dram_parameter("owT", [H * D, HID], BF16, isOutput=False)
    ddm = nc.declare_dram_parameter("ddm", [G, 128, 256], BF16, isOutput=False)
    qdm = nc.declare_dram_parameter("qdm", [128, G, BS], BF16, isOutput=False)
    kdm = nc.declare_dram_parameter("kdm", [128, G, 128], BF16, isOutput=False)
    bdm = nc.declare_dram_parameter("bdm", [128, G], F32, isOutput=False)
    swm = nc.declare_dram_parameter("swm", [128, G * NC], F32, isOutput=False)
    nw = nc.declare_dram_parameter("nw", [128, 16], F32, isOutput=False)
    out = nc.declare_dram_parameter("out", [T, HID], F32, isOutput=True)

    # collective split in two group-halves so the first AllGather hides
    # under the tail of pass 1
    eloc_a = nc.dram_tensor("eloc_a", [2, GSPLIT, D, D], BF16)
    eloc_b = nc.dram_tensor("eloc_b", [2, G - GSPLIT, D, D], BF16)
    egath_a = nc.dram_tensor("egath_a", [NC, 2, GSPLIT, D, D], BF16, addr_space="Shared")
    egath_b = nc.dram_tensor("egath_b", [NC, 2, G - GSPLIT, D, D], BF16, addr_space="Shared")
    ssq_rt = nc.dram_tensor("ssq_rt", [T], F32) if not USE_PNS else None

    with tile.TileContext(nc, pool_alloc_mode="stack") as tc:
        # ---- persistent activations / constants -------------------------
        # xT DMAs go first so phase V starts ASAP; split across the
        # gpsimd (SWDGE) and scalar (Act HWDGE) queues for bandwidth.
        _xt_ctx = ExitStack()
        xt_pool = _xt_ctx.enter_context(tc.tile_pool(name="xt_pool", bufs=1))
        xT = xt_pool.tile([128, KC, T], BF16, name="xT")
        if USE_ACT_QUEUE:
            # 16 fine chunks alternating across both queues: the first
            # k-chunk lands in ~7us instead of ~14, so phase V's first
            # matmul fires that much earlier
            for q16 in range(16):
                eng = nc.gpsimd if q16 % 2 == 0 else nc.scalar
                eng.dma_start(
                    xT[:, q16, :],
                    hsT[q16 * 128 : (q16 + 1) * 128, :].rearrange(
                        "(k p) t -> p k t", p=128
                    )[:, 0, :],
                )
        else:
            for q4 in range(4):
                nc.gpsimd.dma_start(
                    xT[:, q4 * 4 : (q4 + 1) * 4, :],
                    hsT[q4 * 512 : (q4 + 1) * 512, :].rearrange(
                        "(k p) t -> p k t", p=128
                    ),
                )

        _c_ctx = ExitStack()
        c_pool = _c_ctx.enter_context(tc.tile_pool(name="c_pool", bufs=1))
        ident = c_pool.tile([128, 128], BF16, name="ident")
        make_identity(nc, ident[:])
        ones = c_pool.tile([128, 1], BF16, name="ones")
        nc.vector.memset(ones[:], 1.0)
        eps_sb = c_pool.tile([128, 1], F32, name="eps_sb")
        nc.vector.memset(eps_sb[:], EPS)
        kdm_sb = c_pool.tile([128, G, 128], BF16, name="kdm_sb")
        bdm_sb = c_pool.tile([128, G, 1], F32, name="bdm_sb")
        # swm_sb[p, cc, g, 0] = sw[2g + p//64, cc]
        swm_sb = c_pool.tile([128, NC, G, 1], F32, name="swm_sb")
        nw_sb = c_pool.tile([128, 16], F32, name="nw_sb")
        qd_sb = c_pool.tile([128, G, BS], BF16, name="qd_sb")
        # constants ride gpsimd behind the xT chunks: sync starts on the
        # wv stream immediately so phase V's first matmul fires early
        # (moving them to the scalar queue measured slower: it starves
        # the xT odd chunks during the phase-V startup crunch)
        nc.gpsimd.dma_start(kdm_sb[:], kdm[:])
        nc.gpsimd.dma_start(bdm_sb[:, :, 0], bdm[:])
        nc.gpsimd.dma_start(
            swm_sb[:, :, :, 0], swm.rearrange("p (c g) -> p c g", c=NC)
        )
        nc.gpsimd.dma_start(nw_sb[:], nw[:])
        nc.gpsimd.dma_start(qd_sb[:], qdm[:])

        _q_ctx = ExitStack()
        q_pool = _q_ctx.enter_context(tc.tile_pool(name="q_pool", bufs=1))
        qT_sb = q_pool.tile([128, G, T], BF16, name="qT_sb")
        _ys_ctx = ExitStack()
        ys_pool = _ys_ctx.enter_context(tc.tile_pool(name="ys_pool", bufs=1))
        ys_sb = ys_pool.tile([128, G, T], BF16, name="ys_sb")
        _ce_ctx = ExitStack()
        ce_pool = _ce_ctx.enter_context(tc.tile_pool(name="ce_pool", bufs=1))
        c_sb = ce_pool.tile([128, G, NBLK, D], BF16, name="c_sb")
        E_sb = ce_pool.tile([128, G, D], BF16, name="E_sb")
        # weight pool sits below v_pool on the stack so its DMAs carry
        # no memory-reuse dependency on the previous phase's consumers
        _gw_ctx = ExitStack()
        gw_p = _gw_ctx.enter_context(tc.tile_pool(name="gw_p", bufs=3))
        # ow_p opens here (not at phase F) so its SBUF region never
        # aliases the merged-phase pools: the first ow loads would
        # otherwise wait on the last pass2 group's vector tail
        _ow_ctx = ExitStack()
        ow_p = _ow_ctx.enter_context(tc.tile_pool(name="ow_p", bufs=4))
        # v_pool opened last among persistents: it is the only one released
        # mid-stream (stack allocator frees LIFO only)
        _v_ctx = ExitStack()
        v_pool = _v_ctx.enter_context(tc.tile_pool(name="v_pool", bufs=1))
        V_sb = v_pool.tile([128, NCH, H * D], BF16, name="V_sb")

        # pass-1 weight pools open before phase V so the first groups'
        # q/k weights and dd masks prefetch during the V projection
        # (kills the V->pass1 PE gap + HAM rethrottle)
        _p1_ctx = ExitStack()
        wqk_p = _p1_ctx.enter_context(tc.tile_pool(name="wqk_p", bufs=4))
        dd_p = _p1_ctx.enter_context(tc.tile_pool(name="dd_p", bufs=3))
        kT2_hist = {}
        dd_hist = {}
        wqk_pref = {}
        # q/k weights and dd stream on their own queue so they are
        # never head-of-line blocked behind the wv->gw->ow stream on sync
        wq_eng = nc.scalar if USE_ACT_QUEUE else nc.sync
        # high groups first: their AllGather fires after 4 iterations
        # and completes long before the merged phase needs their
        # chunk-start states; the low-group AllGather (end of pass 1)
        # hides under the first merged-phase gate chunks
        g_order = list(range(GSPLIT, G)) + list(range(GSPLIT))

        def prefetch_wqk(g, eng=None):
            eng = eng or wq_eng
            wqk_a = wqk_p.tile([128, 8, 256], BF16, name="wqk_t")
            eng.dma_start(
                wqk_a[:],
                wqkT[0:1024, g, :].rearrange("(ko p) c -> p ko c", p=128),
            )
            wqk_b = wqk_p.tile([128, 8, 256], BF16, name="wqk_t")
            eng.dma_start(
                wqk_b[:],
                wqkT[1024:2048, g, :].rearrange("(ko p) c -> p ko c", p=128),
            )
            dd_g = dd_p.tile([128, 256], BF16, name="dd_g")
            eng.dma_start(dd_g[:], ddm[g])
            dd_hist[g] = dd_g
            wqk_pref[g] = (wqk_a, wqk_b)

        # ---- phase V: value projection (tok-major, all heads) -----------
        with tc.tile_pool(name="wv_p", bufs=6) as wv_p, tc.tile_pool(
            name="ps_v", bufs=1, space="PSUM"
        ) as ps_v:
            for n in range(4):
                # slip the first pass-1 weight prefetches into the wv
                # stream after the first n-passes: early enough to beat
                # pass 1, late enough not to delay the first wv tiles
                if n in (1, 2):
                    prefetch_wqk(g_order[n - 1], nc.sync)
                pv = [
                    ps_v.tile([128, 512], F32, name=f"pv{m}") for m in range(NCH)
                ]
                for k2 in range(KC // 2):
                    wv_t = wv_p.tile([128, 2, 512], BF16, name="wv_t")
                    nc.sync.dma_start(
                        wv_t[:],
                        wvT[
                            k2 * 256 : (k2 + 1) * 256, n * 512 : (n + 1) * 512
                        ].rearrange("(ko p) c -> p ko c", p=128),
                    )
                    for kk in range(2):
                        k = 2 * k2 + kk
                        for m in range(NCH):
                            nc.tensor.matmul(
                                pv[m][:],
                                xT[:, k, m * 128 : (m + 1) * 128],
                                wv_t[:, kk, :],
                                start=(k == 0),
                                stop=(k == KC - 1),
                            )
                            if k == KC - 1:
                                # silu(m) right after m's final matmul so
                                # the 8 activations overlap the remaining
                                # matmuls instead of trailing the n-pass
                                nc.scalar.activation(
                                    V_sb[:, m, n * 512 : (n + 1) * 512],
                                    pv[m][:],
                                    AF.Silu,
                                )

        # ---- pass 1: lag-1 pipeline over head pairs ---------------------
        # iteration i: projections of group i, then intra-attention of
        # group i-1 (whose scalar/vector prerequisites completed during
        # group i's projection matmuls).
        with tc.tile_pool(name="kt_p", bufs=2) as kt_p, tc.tile_pool(
            name="ktok_p", bufs=2
        ) as ktok_p, tc.tile_pool(name="vkd_p", bufs=2) as vkd_p, tc.tile_pool(
            name="awm_p", bufs=3
        ) as awm_p, tc.tile_pool(name="ps1", bufs=1, space="PSUM") as ps1:

            def gen_proj(g):
                """q/k projections of group g, yielding after each 4-matmul
                chunk (16 yields) so intra work of the previous group can
                interleave into the PE stream."""
                if g not in wqk_pref:
                    prefetch_wqk(g)
                wqk_a, wqk_b = wqk_pref.pop(g)

                kT2 = kt_p.tile([128, T], BF16, name="kT2")
                kT2_hist[g] = kT2
                for qk in range(2):
                    cs = slice(qk * 128, qk * 128 + 128)
                    for half in range(2):
                        pp = ps1.tile([128, 512], F32, name="pp", tag="proj", bufs=2)
                        for k4 in range(4):
                            for kk in range(4):
                                k = 4 * k4 + kk
                                wt = wqk_a if k < 8 else wqk_b
                                nc.tensor.matmul(
                                    pp[:],
                                    wt[:, k % 8, cs],
                                    xT[:, k, half * 512 : (half + 1) * 512],
                                    start=(k == 0),
                                    stop=(k == KC - 1),
                                )
                            yield
                        if qk == 0:
                            nc.scalar.activation(
                                qT_sb[:, g, half * 512 : (half + 1) * 512],
                                pp[:],
                                AF.Silu,
                            )
                        else:
                            nc.scalar.activation(
                                kT2[:, half * 512 : (half + 1) * 512], pp[:], AF.Silu
                            )

            def gen_intra(g):
                """intra-attention of group g as a sequence of small PE
                bursts, yielding between them; every scalar/vector round
                trip is covered by a projection chunk of the next group."""
                kT2 = kT2_hist.pop(g)
                dd_g = dd_hist.pop(g)
                # v scaled by key-decay (one mul per block, both heads)
                v_kd = vkd_p.tile([128, NCH, 128], BF16, name="v_kd")
                for m in range(NCH):
                    nc.vector.tensor_mul(
                        v_kd[:, m, :],
                        V_sb[:, m, g * 128 : (g + 1) * 128],
                        kdm_sb[:, g, :],
                    )
                # k -> tok-major via the HWDGE transpose engine: no PE
                # columns, no PSUM tile, no scalar copies. A full proj
                # group of slack covers the DMA latency.
                k_tok = ktok_p.tile([128, NCH, 128], BF16, name="k_tok")
                if USE_DMA_TR:
                    for m in range(NCH):
                        wq_eng.dma_start_transpose(
                            k_tok[:, m, :], kT2[:, m * 128 : (m + 1) * 128]
                        )
                    yield
                    yield
                else:
                    for b2 in range(2):
                        ptr = ps1.tile(
                            [128, 4, 128], BF16, name="ptr", tag="sm", bufs=3
                        )
                        for ch in range(4):
                            cc = b2 * 4 + ch
                            nc.tensor.matmul(
                                ptr[:, ch, :],
                                kT2[:, cc * 128 : (cc + 1) * 128],
                                ident[:],
                                is_transpose=True,
                                skip_group_check=True,
                            )
                        nc.scalar.copy(
                            k_tok[:, b2 * 4 : (b2 + 1) * 4, :].rearrange(
                                "p a b -> p (a b)"
                            ),
                            ptr[:].rearrange("p a b -> p (a b)"),
                        )
                        yield
                # block contributions C_m = k_tok^T (kd*v), both heads at once
                for b2 in range(2):
                    pc4 = ps1.tile([128, 4, 128], F32, name="pc4", tag="sm", bufs=3)
                    for ch in range(4):
                        m = b2 * 4 + ch
                        nc.tensor.matmul(
                            pc4[:, ch, :],
                            k_tok[:, m, :],
                            v_kd[:, m, :],
                            start=True,
                            stop=True,
                            skip_group_check=True,
                        )
                    for hh in range(2):
                        sl = slice(hh * 64, (hh + 1) * 64)
                        nc.scalar.copy(
                            c_sb[sl, g, b2 * 4 : (b2 + 1) * 4, :],
                            pc4[sl, :, hh * 64 : (hh + 1) * 64],
                        )
                    yield
                # qk^T pairs (concurrent on distinct PE row groups; ring
                # slot rotation keeps each pair in distinct PSUM banks),
                # then av pairs two blocks behind
                awms = {}
                for m in range(NBLK + 2):
                    if m < NBLK:
                        paws = []
                        for hh in range(2):
                            hs = slice(hh * 64, (hh + 1) * 64)
                            paw = ps1.tile(
                                [128, 512], F32, name="paw", tag="aw", bufs=2
                            )
                            nc.tensor.matmul(
                                paw[:, 0:128],
                                kT2[hs, m * 128 : (m + 1) * 128],
                                qT_sb[hs, g, m * 128 : (m + 1) * 128],
                                start=True,
                                stop=True,
                                skip_group_check=True,
                            )
                            paws.append(paw)
                        awm = awm_p.tile([128, 2, 128], BF16, name="awm")
                        for hh in range(2):
                            nc.vector.tensor_mul(
                                awm[:, hh, :],
                                paws[hh][:, 0:128],
                                dd_g[:, hh * 128 : (hh + 1) * 128],
                            )
                        awms[m] = awm
                    if m >= 2:
                        mp = m - 2
                        awm_p_tile = awms.pop(mp)
                        pys = ps1.tile([128, 128], F32, name="pys", tag="sm", bufs=3)
                        for hh in range(2):
                            nc.tensor.matmul(
                                pys[hh * 64 : (hh + 1) * 64, :],
                                V_sb[
                                    :, mp, g * 128 + hh * 64 : g * 128 + (hh + 1) * 64
                                ],
                                awm_p_tile[:, hh, :],
                                start=True,
                                stop=True,
                                skip_group_check=True,
                            )
                        nc.scalar.copy(
                            ys_sb[:, g, mp * 128 : (mp + 1) * 128], pys[:]
                        )
                    yield
                # chunk summary E = sum_m bd^(NBLK-1-m) C_m  (vector only)
                nc.vector.tensor_copy(E_sb[:, g, :], c_sb[:, g, 0, :])
                for m in range(1, NBLK):
                    nc.vector.scalar_tensor_tensor(
                        E_sb[:, g, :],
                        E_sb[:, g, :],
                        bdm_sb[:, g, :],
                        c_sb[:, g, m, :],
                        ALU.mult,
                        ALU.add,
                    )
                # split collective: the high groups (processed first in
                # pass 1) gather early, the low groups at the end. eloc
                # writes ride gpsimd where the collective lives, never
                # blocking a weight stream.
                if g == GSPLIT - 1 or g == G - 1:
                    eh, gh, lo, ln = (
                        (eloc_a, egath_a, 0, GSPLIT)
                        if g < G - 1
                        else (eloc_b, egath_b, GSPLIT, G - GSPLIT)
                    )
                    nc.gpsimd.dma_start(
                        eh[0].rearrange("g d e -> d g e"),
                        E_sb[0:64, lo : lo + ln, :],
                    )
                    nc.gpsimd.dma_start(
                        eh[1].rearrange("g d e -> d g e"),
                        E_sb[64:128, lo : lo + ln, :],
                    )
                    nc.gpsimd.collective_compute(
                        "AllGather",
                        ALU.bypass,
                        replica_groups=[list(range(NC))],
                        ins=[eh[:]],
                        outs=[gh[:]],
                    )

            for i in range(G + 1):
                gens = []
                if i < G:
                    gens.append(gen_proj(g_order[i]))
                if i >= 1:
                    gens.append(gen_intra(g_order[i - 1]))
                active = list(gens)
                while active:
                    for gn in list(active):
                        try:
                            next(gn)
                        except StopIteration:
                            active.remove(gn)
                # prefetch two groups ahead; safe only now that group
                # g_order[i]'s consumers are emitted (ring slot reuse)
                if i + 2 < G:
                    prefetch_wqk(g_order[i + 2])

        _p1_ctx.close()
        _v_ctx.close()

        # ---- merged phase: gate proj + pass 2 (inter) + rmsnorm prep ----
        _g_ctx = ExitStack()
        g_pool = _g_ctx.enter_context(tc.tile_pool(name="g_pool", bufs=1))
        gate_sb = g_pool.tile([128, G, T], BF16, name="gate_sb")
        # ss0[:, g, :] = chunk-start state (block 0) for pair g
        ss0 = g_pool.tile([128, G, D], BF16, name="ss0")
        acc = g_pool.tile([128, T], F32, name="acc")
        ns_t = g_pool.tile([128, NCH], F32, name="ns_t")
        ns_sb = g_pool.tile([128, NCH], F32, name="ns_sb")

        with tc.tile_pool(name="egc_p", bufs=2) as egc_p, tc.tile_pool(
            name="egt_p", bufs=2
        ) as egt_p, tc.tile_pool(name="ssg_p", bufs=2) as ssg_p, tc.tile_pool(
            name="tmp_p", bufs=2
        ) as tmp_p, tc.tile_pool(name="sq_p2", bufs=2) as sq_p2, tc.tile_pool(
            name="ps_g", bufs=1, space="PSUM"
        ) as ps_g, tc.tile_pool(name="ps2", bufs=1, space="PSUM") as ps2, tc.tile_pool(
            name="ps_sq", bufs=1, space="PSUM"
        ) as ps_sq:

            def eg_accum(hf):
                # gpsimd (SWDGE) queue: waits on the collective without
                # blocking the gate-weight stream on the sync queue
                gh, lo, ln = (
                    (egath_a, 0, GSPLIT)
                    if hf == 0
                    else (egath_b, GSPLIT, G - GSPLIT)
                )
                gs = slice(lo, lo + ln)
                for cc in range(NC):
                    egc = egc_p.tile([128, ln, D], BF16, name="egc")
                    nc.gpsimd.dma_start(
                        egc[0:64], gh[cc, 0].rearrange("g d e -> d g e")
                    )
                    nc.gpsimd.dma_start(
                        egc[64:128], gh[cc, 1].rearrange("g d e -> d g e")
                    )
                    if cc == 0:
                        nc.vector.tensor_mul(
                            ss0[:, gs, :],
                            egc[:],
                            swm_sb[:, 0, gs, :].broadcast_to([128, ln, D]),
                        )
                    else:
                        egt = egt_p.tile([128, ln, D], BF16, name="egt")
                        nc.vector.tensor_mul(
                            egt[:],
                            egc[:],
                            swm_sb[:, cc, gs, :].broadcast_to([128, ln, D]),
                        )
                        nc.vector.tensor_add(ss0[:, gs, :], ss0[:, gs, :], egt[:])

            # high groups' gather completed mid-pass-1; accumulate their
            # chunk-start states first so pass2 can start immediately
            eg_accum(1)

            def pass2_partA(g, first=False):
                """inter-block term + rmsnorm accumulation for group g —
                no dependency on group g's gate chunk, so it runs one gate
                chunk AHEAD and its vector/scalar tail hides under gate
                matmuls."""
                # per-block chunk-start states via the bd recurrence
                ss_g = ssg_p.tile([128, NBLK, D], BF16, name="ss_g")
                nc.vector.tensor_copy(ss_g[:, 0, :], ss0[:, g, :])
                for m in range(1, NBLK):
                    nc.vector.scalar_tensor_tensor(
                        ss_g[:, m, :],
                        ss_g[:, m - 1, :],
                        bdm_sb[:, g, :],
                        c_sb[:, g, m - 1, :],
                        ALU.mult,
                        ALU.add,
                    )
                pin = ps2.tile([128, NBLK, BS], F32, name="pin", bufs=2)
                for m in range(NBLK):
                    for hh in range(2):
                        hs = slice(hh * 64, (hh + 1) * 64)
                        nc.tensor.matmul(
                            pin[hs, m, :],
                            ss_g[hs, m, :],
                            qT_sb[hs, g, m * BS : (m + 1) * BS],
                            start=True,
                            stop=True,
                            skip_group_check=True,
                        )
                tmp3 = tmp_p.tile([128, NBLK, BS], BF16, name="tmp3")
                nc.vector.tensor_mul(
                    tmp3[:],
                    pin[:],
                    qd_sb[:, g : g + 1, :].broadcast_to([128, NBLK, BS]),
                )
                nc.vector.tensor_add(
                    ys_sb[:, g, :],
                    ys_sb[:, g, :],
                    tmp3[:].rearrange("p a b -> p (a b)"),
                )
                # rmsnorm prep: accumulate sum over groups of ys^2 in SBUF
                sq = sq_p2.tile([128, T], F32, name="sq")
                nc.scalar.activation(sq[:], ys_sb[:, g, :], AF.Square)
                if first:
                    nc.vector.tensor_copy(acc[:], sq[:])
                else:
                    nc.vector.tensor_add(acc[:], acc[:], sq[:])

            def pass2_partB(g):
                # gate <- ys * norm_w * gate; needs group g's sigmoid.
                # With phase F consuming the last gate chunks last, this
                # never gates the PE.
                nc.vector.scalar_tensor_tensor(
                    gate_sb[:, g, :],
                    ys_sb[:, g, :],
                    nw_sb[:, g : g + 1],
                    gate_sb[:, g, :],
                    ALU.mult,
                    ALU.mult,
                )

            # partA runs one gate chunk ahead of partB; by the last gate
            # chunk every rmsnorm contribution is in, so the finalize
            # (emitted after partA of the last group) hides under it and
            # only two cheap gate-STTs trail the final gate matmuls
            g_order = list(range(GSPLIT, G)) + list(range(GSPLIT))
            for i in range(G):
                gm = g_order[i]
                gw_t = gw_p.tile([128, KC, 128], BF16, name="gw_t")
                nc.sync.dma_start(
                    gw_t[:],
                    gwT[:, gm * 128 : (gm + 1) * 128].rearrange(
                        "(ko p) c -> p ko c", p=128
                    ),
                )
                for gn in range(2):
                    pg = ps_g.tile([128, 512], F32, name="pg", bufs=2)
                    for k in range(KC):
                        nc.tensor.matmul(
                            pg[:],
                            gw_t[:, k, :],
                            xT[:, k, gn * 512 : (gn + 1) * 512],
                            start=(k == 0),
                            stop=(k == KC - 1),
                        )
                    nc.scalar.activation(
                        gate_sb[:, gm, gn * 512 : (gn + 1) * 512],
                        pg[:],
                        AF.Sigmoid,
                    )
                if i == 5:
                    eg_accum(0)
                if i >= 1:
                    pass2_partB(g_order[i - 1])
                if i == 0:
                    pass2_partA(g_order[0], first=True)
                    pass2_partA(g_order[1])
                elif i + 1 < G:
                    pass2_partA(g_order[i + 1])
            pass2_partB(g_order[G - 1])

            # rmsnorm scale per token: reduce acc over partitions
            acc_bf = sq_p2.tile([128, NCH, 128], BF16, name="acc_bf")
            nc.scalar.copy(
                acc_bf[:].rearrange("p a b -> p (a b)"), acc[:]
            )
            if USE_PNS:
                # 8 tiny matmuls (stationary = token chunk, moving = ones)
                # -> [128 tok, 1] per chunk, the layout phase F needs
                pns = ps_sq.tile([128, NCH], F32, name="pns")
                for m in range(NCH):
                    nc.tensor.matmul(
                        pns[:, m : m + 1],
                        acc_bf[:, m, :],
                        ones[:],
                        start=True,
                        stop=True,
                        skip_group_check=True,
                    )
                nc.scalar.activation(
                    ns_t[:], pns[:], AF.Sqrt, bias=eps_sb[:, 0:1], scale=1.0 / (H * D)
                )
                nc.vector.reciprocal(ns_sb[:], ns_t[:])
            else:
                # baseline-style: ones^T @ acc -> [1, T], DRAM roundtrip to
                # redistribute tokens across partitions
                ssq = ps_sq.tile([1, T], F32, name="ssq")
                for half in range(2):
                    nc.tensor.matmul(
                        ssq[:, half * 512 : (half + 1) * 512],
                        ones[:],
                        acc_bf[:].rearrange("p a b -> p (a b)")[
                            :, half * 512 : (half + 1) * 512
                        ],
                        start=True,
                        stop=True,
                        skip_group_check=True,
                    )
                ssq_sb = sq_p2.tile([1, T], F32, name="ssq_sb")
                nc.vector.tensor_copy(ssq_sb[:], ssq[:])
                nc.sync.dma_start(ssq_rt[:], ssq_sb[:])
                ns_l = sq_p2.tile([128, NCH], F32, name="ns_l")
                nc.sync.dma_start(ns_l[:], ssq_rt.rearrange("(c p) -> p c", p=128))
                nc.scalar.activation(
                    ns_t[:], ns_l[:], AF.Sqrt, bias=eps_sb[:, 0:1], scale=1.0 / (H * D)
                )
                nc.vector.reciprocal(ns_sb[:], ns_t[:])

        # ---- phase F: output projection --------------------------------
        with tc.tile_pool(
            name="oo_p", bufs=1
        ) as oo_p, tc.tile_pool(name="ps_o", bufs=1, space="PSUM") as ps_o:
            # consume gate chunks in the order pass 2 finishes them
            # (12..15 first, 11 last) so the first n-pass never waits on
            # the final pass2 group's vector tail
            k2_order = [6, 7, 0, 1, 2, 3, 4, 5]
            for n in range(4):
                po = [
                    ps_o.tile([128, 512], F32, name=f"po{m}") for m in range(NCH)
                ]
                for ki, k2 in enumerate(k2_order):
                    ow_t = ow_p.tile([128, 2, 512], BF16, name="ow_t")
                    # gpsimd queue: idle by now, so ow never waits behind
                    # the gw triggers' buffer-free stalls on sync
                    nc.gpsimd.dma_start(
                        ow_t[:],
                        owT[
                            k2 * 256 : (k2 + 1) * 256, n * 512 : (n + 1) * 512
                        ].rearrange("(ko p) c -> p ko c", p=128),
                    )
                    for kk in range(2):
                        k = 2 * k2 + kk
                        for m in range(NCH):
                            nc.tensor.matmul(
                                po[m][:],
                                gate_sb[:, k, m * 128 : (m + 1) * 128],
                                ow_t[:, kk, :],
                                start=(ki == 0 and kk == 0),
                                stop=(ki == len(k2_order) - 1 and kk == 1),
                            )
                oo_all = oo_p.tile([128, NCH, 512], F32, name="oo_all")
                for mh in range(4):
                    for mm in range(2):
                        m = 2 * mh + mm
                        nc.vector.tensor_scalar_mul(
                            oo_all[:, m, :], po[m][:], ns_sb[:, m : m + 1]
                        )
                    # out writes alternate across idle queues so the final
                    # writes never back up behind one another
                    if USE_ACT_QUEUE:
                        oq = nc.scalar if mh % 2 == 0 else nc.sync
                    else:
                        oq = nc.sync if mh % 2 == 0 else nc.gpsimd
                    oq.dma_start(
                        out[
                            mh * 256 : (mh + 1) * 256,
                            n * 512 : (n + 1) * 512,
                        ].rearrange("(m p) c -> p m c", p=128),
                        oo_all[:, mh * 2 : (mh + 1) * 2, :],
                    )
        _g_ctx.close()
        _ow_ctx.close()
        _gw_ctx.close()
        _ce_ctx.close()
        _ys_ctx.close()
        _q_ctx.close()
        _c_ctx.close()
        _xt_ctx.close()
    nc.finalize()
    return nc


_CACHE = {}


def _get_nc():
    if "nc" not in _CACHE:
        _CACHE["nc"] = _build_nc()
    return _CACHE["nc"]


def _host_prep(hidden_states, qkv_w, out_w, gate_w, norm_w):
    slope, qd, kd, dd, bd = _decay()
    w3 = qkv_w.reshape(H, 3 * D, HID)
    wq = w3[:, 0:D, :].reshape(H * D, HID)
    wk = w3[:, D : 2 * D, :].reshape(H * D, HID)
    wv = w3[:, 2 * D : 3 * D, :].reshape(H * D, HID)
    # wqkT[:, g, 0:128] = q dims of heads 2g,2g+1; [:, g, 128:256] = k dims
    wqkT = np.concatenate(
        [
            np.ascontiguousarray(wq.T).reshape(HID, G, 128),
            np.ascontiguousarray(wk.T).reshape(HID, G, 128),
        ],
        axis=2,
    ).astype(BF)
    wvT = np.ascontiguousarray(wv.T).astype(BF)
    gwT = np.ascontiguousarray(gate_w.T).astype(BF)
    owT = np.ascontiguousarray(out_w.T).astype(BF)

    # ddm[g, p, hh*128 + i] = dd[2g+hh, i, p]   (p = key token j)
    dd_t = dd.transpose(0, 2, 1)  # (H, j, i)
    ddm = np.ascontiguousarray(
        dd_t.reshape(G, 2, 128, BS).transpose(0, 2, 1, 3).reshape(G, 128, 256)
    ).astype(BF)
    # qdm[p, g, i] = qd[2g + p//64, i]
    qdm = np.ascontiguousarray(
        np.broadcast_to(qd.reshape(G, 2, 1, BS), (G, 2, 64, BS))
        .transpose(1, 2, 0, 3)
        .reshape(128, G, BS)
    ).astype(BF)
    # kdm[p, g, hh*64+d] = kd[2g+hh, p]
    kdm = np.ascontiguousarray(
        np.broadcast_to(
            kd.T.reshape(128, G, 2, 1), (128, G, 2, 64)
        ).reshape(128, G, 128)
    ).astype(BF)
    # bdm[p, g] = bd[2g + p//64]
    bdm = np.ascontiguousarray(
        np.broadcast_to(bd.reshape(G, 2, 1), (G, 2, 64)).transpose(1, 2, 0).reshape(128, G)
    ).astype(np.float32)
    nw = np.ascontiguousarray(norm_w.reshape(16, 128).T).astype(np.float32)

    shared = dict(wqkT=wqkT, wvT=wvT, gwT=gwT, owT=owT, ddm=ddm, qdm=qdm,
                  kdm=kdm, bdm=bdm, nw=nw)
    in_maps = []
    for c in range(NC):
        bb, p = c // 4, c % 4
        hsT = np.ascontiguousarray(
            hidden_states[bb, p * T : (p + 1) * T, :].T
        ).astype(BF)
        sw = np.zeros((H, NC), dtype=np.float64)
        for cc in range(NC):
            if cc // 4 == bb and cc % 4 < p:
                sw[:, cc] = bd ** (NBLK * (p - 1 - (cc % 4)))
        # swm[p_, cc*G+g] = sw[2g + p_//64, cc]  (cc-major)
        swm = np.ascontiguousarray(
            np.broadcast_to(sw.reshape(G, 2, 1, NC), (G, 2, 64, NC))
            .transpose(1, 2, 3, 0)
            .reshape(128, NC * G)
        ).astype(np.float32)
        in_maps.append(dict(hsT=hsT, swm=swm, **shared))
    return in_maps


def _run(inputs, trace=False):
    nc = _get_nc()
    in_maps = _host_prep(
        np.asarray(inputs["hidden_states"], dtype=np.float32),
        np.asarray(inputs["qkv_w"], dtype=np.float32),
        np.asarray(inputs["out_w"], dtype=np.float32),
        np.asarray(inputs["gate_w"], dtype=np.float32),
        np.asarray(inputs["norm_w"], dtype=np.float32),
    )
    res = run_bass_kernel_spmd(nc, in_maps, core_ids=list(range(NC)), trace=trace)
    full = np.empty((B, S, HID), dtype=np.float32)
    for c in range(NC):
        bb, p = c // 4, c % 4
        full[bb, p * T : (p + 1) * T, :] = res.results[c]["out"]
    return full, res


def kernel(**inputs):
    return _run(inputs, trace=False)[0]


def kernel_traced(**inputs):
    full, res = _run(inputs, trace=True)
    return full, res.exec_time_ns


# revision 66
# speedup vs baseline: 1.0323x; 1.0323x over previous
"""MiniMax Lightning Attention on 8 Trainium2 NeuronCores.

Sharding: sequence-parallel. Core c handles batch c//4, token chunk
(c%4)*1024..+1024. The decay recurrence crosses chunk boundaries; each
core computes its local per-chunk decay-weighted KV summary E, an
AllGather shares the 8 summaries, and each core reconstructs its
chunk-start state as a decay-weighted sum.

The global decay attention is block-decomposed at B=128 (exact for any
block size): intra-block masked attention + inter-block recurrent state.
B=128 halves the intra-attention PE columns vs B=256 and aligns blocks
with 128-partition tiles.

Fused single-residency design: all big activations (x, V, q, ys, gate)
stay SBUF-resident in bf16; weights stream in bf16. Heads are processed
in pairs (dim groups of 128). Pass 1 runs a lag-1 software pipeline:
group g's projections issue before group g-1's intra-attention so the
PE never waits on scalar/vector latency. Head-pair matmuls (64-row or
64-col) are emitted adjacently so they run concurrently on distinct PE
row/col groups.
"""

import numpy as np
import ml_dtypes

from contextlib import ExitStack

import concourse.bacc as bacc
import concourse.mybir as mybir
import concourse.tile as tile
from concourse.bass_utils import run_bass_kernel_spmd
from concourse.masks import make_identity


AF = mybir.ActivationFunctionType
ALU = mybir.AluOpType
F32 = mybir.dt.float32
BF16 = mybir.dt.bfloat16

H = 32
D = 64
BS = 128              # attention block size (exact for any B)
HID = 2048
B = 2
S = 4096
NC = 8
T = S // 4            # tokens per core (1024)
NCH = T // 128        # 8 token chunks of 128 == blocks per core
NBLK = T // BS        # 8 blocks per core
G = H // 2            # 16 head pairs (dim groups of 128)
GSPLIT = 8            # collective split point (groups 0:8 / 8:16)
KC = HID // 128       # 16 contraction chunks
LAYER_IDX = 0
NUM_LAYERS = 32
EPS = 1e-5

BF = ml_dtypes.bfloat16
USE_ACT_QUEUE = True
USE_PNS = True      # rmsnorm partition-reduce via tiny matmuls (vs DRAM roundtrip)
USE_DMA_TR = False  # HWDGE xbar transpose measured much slower than PE


def _decay():
    base = 1.0 / 2.0 ** (8.0 / H)
    rate = base ** (np.arange(H, dtype=np.float64) + 1.0)
    factor = 1.0 - LAYER_IDX / (NUM_LAYERS - 1 + 1e-5) + 1e-5
    slope = rate * factor                                  # (H,)
    r = np.arange(BS, dtype=np.float64) + 1.0
    qd = np.exp(-slope[:, None] * r[None, :])              # (H, BS) query decay
    kd = np.exp(-slope[:, None] * (BS - r[None, :]))       # (H, BS) key decay
    ij = r[:, None] - r[None, :]                           # i - j
    dd = np.where(
        ij[None] >= 0, np.exp(-slope[:, None, None] * ij[None]), 0.0
    )                                                      # (H, BS_i, BS_j)
    bd = np.exp(-slope * BS)                               # (H,) block decay
    return slope, qd, kd, dd, bd


def _build_nc():
    nc = bacc.Bacc(num_devices=NC)
    hsT = nc.declare_dram_parameter("hsT", [HID, T], BF16, isOutput=False)
    wvT = nc.declare_dram_parameter("wvT", [HID, H * D], BF16, isOutput=False)
    wqkT = nc.declare_dram_parameter("wqkT", [HID, G, 256], BF16, isOutput=False)
    gwT = nc.declare_dram_parameter("gwT", [HID, HID], BF16, isOutput=False)
    owT = nc.declare_dram_parameter("owT", [H * D, HID], BF16, isOutput=False)
    ddm = nc.declare_dram_parameter("ddm", [G, 128, 256], BF16, isOutput=False)
    qdm = nc.declare_dram_parameter("qdm", [128, G, BS], BF16, isOutput=False)
    kdm = nc.declare_dram_parameter("kdm", [128, G, 128], BF16, isOutput=False)
    bdm = nc.declare_dram_parameter("bdm", [128, G], F32, isOutput=False)
    swm = nc.declare_dram_parameter("swm", [128, G * NC], F32, isOutput=False)
    nw = nc.declare_dram_parameter("nw", [128, 16], F32, isOutput=False)
    out = nc.declare_dram_parameter("out", [T, HID], F32, isOutput=True)

    # collective split in two group-halves so the first AllGather hides
    # under the tail of pass 1
    eloc_a = nc.dram_tensor("eloc_a", [2, GSPLIT, D, D], BF16)
    eloc_b = nc.dram_tensor("eloc_b", [2, G - GSPLIT, D, D], BF16)
    egath_a = nc.dram_tensor("egath_a", [NC, 2, GSPLIT, D, D], BF16, addr_space="Shared")
    egath_b = nc.dram_tensor("egath_b", [NC, 2, G - GSPLIT, D, D], BF16, addr_space="Shared")
    ssq_rt = nc.dram_tensor("ssq_rt", [T], F32) if not USE_PNS else None

    with tile.TileContext(nc, pool_alloc_mode="stack") as tc:
        # ---- persistent activations / constants -------------------------
        # xT DMAs go first so phase V starts ASAP; split across the
        # gpsimd (SWDGE) and scalar (Act HWDGE) queues for bandwidth.
        _xt_ctx = ExitStack()
        xt_pool = _xt_ctx.enter_context(tc.tile_pool(name="xt_pool", bufs=1))
        xT = xt_pool.tile([128, KC, T], BF16, name="xT")
        if USE_ACT_QUEUE:
            # 16 fine chunks alternating across both queues: the first
            # k-chunk lands in ~7us instead of ~14, so phase V's first
            # matmul fires that much earlier
            for q16 in range(16):
                eng = nc.gpsimd if q16 % 2 == 0 else nc.scalar
                eng.dma_start(
                    xT[:, q16, :],
                    hsT[q16 * 128 : (q16 + 1) * 128, :].rearrange(
                        "(k p) t -> p k t", p=128
                    )[:, 0, :],
                )
        else:
            for q4 in range(4):
                nc.gpsimd.dma_start(
                    xT[:, q4 * 4 : (q4 + 1) * 4, :],
                    hsT[q4 * 512 : (q4 + 1) * 512, :].rearrange(
                        "(k p) t -> p k t", p=128
                    ),
                )

        _c_ctx = ExitStack()
        c_pool = _c_ctx.enter_context(tc.tile_pool(name="c_pool", bufs=1))
        ident = c_pool.tile([128, 128], BF16, name="ident")
        make_identity(nc, ident[:])
        ones = c_pool.tile([128, 1], BF16, name="ones")
        nc.vector.memset(ones[:], 1.0)
        eps_sb = c_pool.tile([128, 1], F32, name="eps_sb")
        nc.vector.memset(eps_sb[:], EPS)
        kdm_sb = c_pool.tile([128, G, 128], BF16, name="kdm_sb")
        bdm_sb = c_pool.tile([128, G, 1], F32, name="bdm_sb")
        # swm_sb[p, cc, g, 0] = sw[2g + p//64, cc]
        swm_sb = c_pool.tile([128, NC, G, 1], F32, name="swm_sb")
        nw_sb = c_pool.tile([128, 16], F32, name="nw_sb")
        qd_sb = c_pool.tile([128, G, BS], BF16, name="qd_sb")
        # constants ride gpsimd behind the xT chunks: sync starts on the
        # wv stream immediately so phase V's first matmul fires early
        # (moving them to the scalar queue measured slower: it starves
        # the xT odd chunks during the phase-V startup crunch)
        nc.gpsimd.dma_start(kdm_sb[:], kdm[:])
        nc.gpsimd.dma_start(bdm_sb[:, :, 0], bdm[:])
        nc.gpsimd.dma_start(
            swm_sb[:, :, :, 0], swm.rearrange("p (c g) -> p c g", c=NC)
        )
        nc.gpsimd.dma_start(nw_sb[:], nw[:])
        nc.gpsimd.dma_start(qd_sb[:], qdm[:])

        _q_ctx = ExitStack()
        q_pool = _q_ctx.enter_context(tc.tile_pool(name="q_pool", bufs=1))
        qT_sb = q_pool.tile([128, G, T], BF16, name="qT_sb")
        _ys_ctx = ExitStack()
        ys_pool = _ys_ctx.enter_context(tc.tile_pool(name="ys_pool", bufs=1))
        ys_sb = ys_pool.tile([128, G, T], BF16, name="ys_sb")
        _ce_ctx = ExitStack()
        ce_pool = _ce_ctx.enter_context(tc.tile_pool(name="ce_pool", bufs=1))
        c_sb = ce_pool.tile([128, G, NBLK, D], BF16, name="c_sb")
        E_sb = ce_pool.tile([128, G, D], BF16, name="E_sb")
        # weight pool sits below v_pool on the stack so its DMAs carry
        # no memory-reuse dependency on the previous phase's consumers
        _gw_ctx = ExitStack()
        gw_p = _gw_ctx.enter_context(tc.tile_pool(name="gw_p", bufs=3))
        # ow_p opens here (not at phase F) so its SBUF region never
        # aliases the merged-phase pools: the first ow loads would
        # otherwise wait on the last pass2 group's vector tail
        _ow_ctx = ExitStack()
        ow_p = _ow_ctx.enter_context(tc.tile_pool(name="ow_p", bufs=4))
        # v_pool opened last among persistents: it is the only one released
        # mid-stream (stack allocator frees LIFO only)
        _v_ctx = ExitStack()
        v_pool = _v_ctx.enter_context(tc.tile_pool(name="v_pool", bufs=1))
        V_sb = v_pool.tile([128, NCH, H * D], BF16, name="V_sb")

        # pass-1 weight pools open before phase V so the first groups'
        # q/k weights and dd masks prefetch during the V projection
        # (kills the V->pass1 PE gap + HAM rethrottle)
        _p1_ctx = ExitStack()
        wqk_p = _p1_ctx.enter_context(tc.tile_pool(name="wqk_p", bufs=4))
        dd_p = _p1_ctx.enter_context(tc.tile_pool(name="dd_p", bufs=3))
        kT2_hist = {}
        dd_hist = {}
        wqk_pref = {}
        # q/k weights and dd stream on their own queue so they are
        # never head-of-line blocked behind the wv->gw->ow stream on sync
        wq_eng = nc.scalar if USE_ACT_QUEUE else nc.sync
        # high groups first: their AllGather fires after 4 iterations
        # and completes long before the merged phase needs their
        # chunk-start states; the low-group AllGather (end of pass 1)
        # hides under the first merged-phase gate chunks
        g_order = list(range(GSPLIT, G)) + list(range(GSPLIT))

        def prefetch_wqk(g, eng=None):
            eng = eng or wq_eng
            wqk_a = wqk_p.tile([128, 8, 256], BF16, name="wqk_t")
            eng.dma_start(
                wqk_a[:],
                wqkT[0:1024, g, :].rearrange("(ko p) c -> p ko c", p=128),
            )
            wqk_b = wqk_p.tile([128, 8, 256], BF16, name="wqk_t")
            eng.dma_start(
                wqk_b[:],
                wqkT[1024:2048, g, :].rearrange("(ko p) c -> p ko c", p=128),
            )
            dd_g = dd_p.tile([128, 256], BF16, name="dd_g")
            eng.dma_start(dd_g[:], ddm[g])
            dd_hist[g] = dd_g
            wqk_pref[g] = (wqk_a, wqk_b)

        # ---- phase V: value projection (tok-major, all heads) -----------
        with tc.tile_pool(name="wv_p", bufs=6) as wv_p, tc.tile_pool(
            name="ps_v", bufs=1, space="PSUM"
        ) as ps_v:
            for n in range(4):
                # slip the first pass-1 weight prefetches into the wv
                # stream after the first n-passes: early enough to beat
                # pass 1, late enough not to delay the first wv tiles
                if n in (1, 2):
                    prefetch_wqk(g_order[n - 1], nc.sync)
                pv = [
                    ps_v.tile([128, 512], F32, name=f"pv{m}") for m in range(NCH)
                ]
                for k2 in range(KC // 2):
                    wv_t = wv_p.tile([128, 2, 512], BF16, name="wv_t")
                    nc.sync.dma_start(
                        wv_t[:],
                        wvT[
                            k2 * 256 : (k2 + 1) * 256, n * 512 : (n + 1) * 512
                        ].rearrange("(ko p) c -> p ko c", p=128),
                    )
                    for kk in range(2):
                        k = 2 * k2 + kk
                        for m in range(NCH):
                            nc.tensor.matmul(
                                pv[m][:],
                                xT[:, k, m * 128 : (m + 1) * 128],
                                wv_t[:, kk, :],
                                start=(k == 0),
                                stop=(k == KC - 1),
                            )
                            if k == KC - 1:
                                # silu(m) right after m's final matmul so
                                # the 8 activations overlap the remaining
                                # matmuls instead of trailing the n-pass
                                nc.scalar.activation(
                                    V_sb[:, m, n * 512 : (n + 1) * 512],
                                    pv[m][:],
                                    AF.Silu,
                                )

        # ---- pass 1: lag-1 pipeline over head pairs ---------------------
        # iteration i: projections of group i, then intra-attention of
        # group i-1 (whose scalar/vector prerequisites completed during
        # group i's projection matmuls).
        with tc.tile_pool(name="kt_p", bufs=2) as kt_p, tc.tile_pool(
            name="ktok_p", bufs=2
        ) as ktok_p, tc.tile_pool(name="vkd_p", bufs=2) as vkd_p, tc.tile_pool(
            name="awm_p", bufs=3
        ) as awm_p, tc.tile_pool(name="ps1", bufs=1, space="PSUM") as ps1:

            def gen_proj(g):
                """q/k projections of group g, yielding after each 4-matmul
                chunk (16 yields) so intra work of the previous group can
                interleave into the PE stream."""
                if g not in wqk_pref:
                    prefetch_wqk(g)
                wqk_a, wqk_b = wqk_pref.pop(g)

                kT2 = kt_p.tile([128, T], BF16, name="kT2")
                kT2_hist[g] = kT2
                for qk in range(2):
                    cs = slice(qk * 128, qk * 128 + 128)
                    for half in range(2):
                        pp = ps1.tile([128, 512], F32, name="pp", tag="proj", bufs=2)
                        for k4 in range(4):
                            for kk in range(4):
                                k = 4 * k4 + kk
                                wt = wqk_a if k < 8 else wqk_b
                                nc.tensor.matmul(
                                    pp[:],
                                    wt[:, k % 8, cs],
                                    xT[:, k, half * 512 : (half + 1) * 512],
                                    start=(k == 0),
                                    stop=(k == KC - 1),
                                )
                            yield
                        if qk == 0:
                            nc.scalar.activation(
                                qT_sb[:, g, half * 512 : (half + 1) * 512],
                                pp[:],
                                AF.Silu,
                            )
                        else:
                            nc.scalar.activation(
                                kT2[:, half * 512 : (half + 1) * 512], pp[:], AF.Silu
                            )

            def gen_intra(g):
                """intra-attention of group g as a sequence of small PE
                bursts, yielding between them; every scalar/vector round
                trip is covered by a projection chunk of the next group."""
                kT2 = kT2_hist.pop(g)
                dd_g = dd_hist.pop(g)
                # v scaled by key-decay (one mul per block, both heads)
                v_kd = vkd_p.tile([128, NCH, 128], BF16, name="v_kd")
                for m in range(NCH):
                    nc.vector.tensor_mul(
                        v_kd[:, m, :],
                        V_sb[:, m, g * 128 : (g + 1) * 128],
                        kdm_sb[:, g, :],
                    )
                # k -> tok-major via the HWDGE transpose engine: no PE
                # columns, no PSUM tile, no scalar copies. A full proj
                # group of slack covers the DMA latency.
                k_tok = ktok_p.tile([128, NCH, 128], BF16, name="k_tok")
                if USE_DMA_TR:
                    for m in range(NCH):
                        wq_eng.dma_start_transpose(
                            k_tok[:, m, :], kT2[:, m * 128 : (m + 1) * 128]
                        )
                    yield
                    yield
                else:
                    for b2 in range(2):
                        ptr = ps1.tile(
                            [128, 4, 128], BF16, name="ptr", tag="sm", bufs=3
                        )
                        for ch in range(4):
                            cc = b2 * 4 + ch
                            nc.tensor.matmul(
                                ptr[:, ch, :],
                                kT2[:, cc * 128 : (cc + 1) * 128],
                                ident[:],
                                is_transpose=True,
                                skip_group_check=True,
                            )
                        nc.scalar.copy(
                            k_tok[:, b2 * 4 : (b2 + 1) * 4, :].rearrange(
                                "p a b -> p (a b)"
                            ),
                            ptr[:].rearrange("p a b -> p (a b)"),
                        )
                        yield
                # block contributions C_m = k_tok^T (kd*v), both heads at once
                for b2 in range(2):
                    pc4 = ps1.tile([128, 4, 128], F32, name="pc4", tag="sm", bufs=3)
                    for ch in range(4):
                        m = b2 * 4 + ch
                        nc.tensor.matmul(
                            pc4[:, ch, :],
                            k_tok[:, m, :],
                            v_kd[:, m, :],
                            start=True,
                            stop=True,
                            skip_group_check=True,
                        )
                    for hh in range(2):
                        sl = slice(hh * 64, (hh + 1) * 64)
                        nc.scalar.copy(
                            c_sb[sl, g, b2 * 4 : (b2 + 1) * 4, :],
                            pc4[sl, :, hh * 64 : (hh + 1) * 64],
                        )
                    yield
                # qk^T pairs (concurrent on distinct PE row groups; ring
                # slot rotation keeps each pair in distinct PSUM banks),
                # then av pairs two blocks behind
                awms = {}
                for m in range(NBLK + 2):
                    if m < NBLK:
                        paws = []
                        for hh in range(2):
                            hs = slice(hh * 64, (hh + 1) * 64)
                            paw = ps1.tile(
                                [128, 512], F32, name="paw", tag="aw", bufs=2
                            )
                            nc.tensor.matmul(
                                paw[:, 0:128],
                                kT2[hs, m * 128 : (m + 1) * 128],
                                qT_sb[hs, g, m * 128 : (m + 1) * 128],
                                start=True,
                                stop=True,
                                skip_group_check=True,
                            )
                            paws.append(paw)
                        awm = awm_p.tile([128, 2, 128], BF16, name="awm")
                        for hh in range(2):
                            nc.vector.tensor_mul(
                                awm[:, hh, :],
                                paws[hh][:, 0:128],
                                dd_g[:, hh * 128 : (hh + 1) * 128],
                            )
                        awms[m] = awm
                    if m >= 2:
                        mp = m - 2
                        awm_p_tile = awms.pop(mp)
                        pys = ps1.tile([128, 128], F32, name="pys", tag="sm", bufs=3)
                        for hh in range(2):
                            nc.tensor.matmul(
                                pys[hh * 64 : (hh + 1) * 64, :],
                                V_sb[
                                    :, mp, g * 128 + hh * 64 : g * 128 + (hh + 1) * 64
                                ],
                                awm_p_tile[:, hh, :],
                                start=True,
                                stop=True,
                                skip_group_check=True,
                            )
                        nc.scalar.copy(
                            ys_sb[:, g, mp * 128 : (mp + 1) * 128], pys[:]
                        )
                    yield
                # chunk summary E = sum_m bd^(NBLK-1-m) C_m  (vector only)
                nc.vector.tensor_copy(E_sb[:, g, :], c_sb[:, g, 0, :])
                for m in range(1, NBLK):
                    nc.vector.scalar_tensor_tensor(
                        E_sb[:, g, :],
                        E_sb[:, g, :],
                        bdm_sb[:, g, :],
                        c_sb[:, g, m, :],
                        ALU.mult,
                        ALU.add,
                    )
                # split collective: the high groups (processed first in
                # pass 1) gather early, the low groups at the end. eloc
                # writes ride gpsimd where the collective lives, never
                # blocking a weight stream.
                if g == GSPLIT - 1 or g == G - 1:
                    eh, gh, lo, ln = (
                        (eloc_a, egath_a, 0, GSPLIT)
                        if g < G - 1
                        else (eloc_b, egath_b, GSPLIT, G - GSPLIT)
                    )
                    nc.gpsimd.dma_start(
                        eh[0].rearrange("g d e -> d g e"),
                        E_sb[0:64, lo : lo + ln, :],
                    )
                    nc.gpsimd.dma_start(
                        eh[1].rearrange("g d e -> d g e"),
                        E_sb[64:128, lo : lo + ln, :],
                    )
                    nc.gpsimd.collective_compute(
                        "AllGather",
                        ALU.bypass,
                        replica_groups=[list(range(NC))],
                        ins=[eh[:]],
                        outs=[gh[:]],
                    )

            for i in range(G + 1):
                gens = []
                if i < G:
                    gens.append(gen_proj(g_order[i]))
                if i >= 1:
                    gens.append(gen_intra(g_order[i - 1]))
                active = list(gens)
                while active:
                    for gn in list(active):
                        try:
                            next(gn)
                        except StopIteration:
                            active.remove(gn)
                # prefetch two groups ahead; safe only now that group
                # g_order[i]'s consumers are emitted (ring slot reuse)
                if i + 2 < G:
                    prefetch_wqk(g_order[i + 2])

        _p1_ctx.close()
        _v_ctx.close()

        # ---- merged phase: gate proj + pass 2 (inter) + rmsnorm prep ----
        _g_ctx = ExitStack()
        g_pool = _g_ctx.enter_context(tc.tile_pool(name="g_pool", bufs=1))
        gate_sb = g_pool.tile([128, G, T], BF16, name="gate_sb")
        # ss0[:, g, :] = chunk-start state (block 0) for pair g
        ss0 = g_pool.tile([128, G, D], BF16, name="ss0")
        acc = g_pool.tile([128, T], F32, name="acc")
        ns_t = g_pool.tile([128, NCH], F32, name="ns_t")
        ns_sb = g_pool.tile([128, NCH], F32, name="ns_sb")

        with tc.tile_pool(name="egc_p", bufs=2) as egc_p, tc.tile_pool(
            name="egt_p", bufs=2
        ) as egt_p, tc.tile_pool(name="ssg_p", bufs=2) as ssg_p, tc.tile_pool(
            name="tmp_p", bufs=2
        ) as tmp_p, tc.tile_pool(name="sq_p2", bufs=2) as sq_p2, tc.tile_pool(
            name="ps_g", bufs=1, space="PSUM"
        ) as ps_g, tc.tile_pool(name="ps2", bufs=1, space="PSUM") as ps2, tc.tile_pool(
            name="ps_sq", bufs=1, space="PSUM"
        ) as ps_sq:

            def eg_accum(hf):
                # gpsimd (SWDGE) queue: waits on the collective without
                # blocking the gate-weight stream on the sync queue
                gh, lo, ln = (
                    (egath_a, 0, GSPLIT)
                    if hf == 0
                    else (egath_b, GSPLIT, G - GSPLIT)
                )
                gs = slice(lo, lo + ln)
                for cc in range(NC):
                    egc = egc_p.tile([128, ln, D], BF16, name="egc")
                    nc.gpsimd.dma_start(
                        egc[0:64], gh[cc, 0].rearrange("g d e -> d g e")
                    )
                    nc.gpsimd.dma_start(
                        egc[64:128], gh[cc, 1].rearrange("g d e -> d g e")
                    )
                    if cc == 0:
                        nc.vector.tensor_mul(
                            ss0[:, gs, :],
                            egc[:],
                            swm_sb[:, 0, gs, :].broadcast_to([128, ln, D]),
                        )
                    else:
                        egt = egt_p.tile([128, ln, D], BF16, name="egt")
                        nc.vector.tensor_mul(
                            egt[:],
                            egc[:],
                            swm_sb[:, cc, gs, :].broadcast_to([128, ln, D]),
                        )
                        nc.vector.tensor_add(ss0[:, gs, :], ss0[:, gs, :], egt[:])

            # high groups' gather completed mid-pass-1; accumulate their
            # chunk-start states first so pass2 can start immediately
            eg_accum(1)

            def pass2_partA(g, first=False):
                """inter-block term + rmsnorm accumulation for group g —
                no dependency on group g's gate chunk, so it runs one gate
                chunk AHEAD and its vector/scalar tail hides under gate
                matmuls."""
                # per-block chunk-start states via the bd recurrence
                ss_g = ssg_p.tile([128, NBLK, D], BF16, name="ss_g")
                nc.vector.tensor_copy(ss_g[:, 0, :], ss0[:, g, :])
                for m in range(1, NBLK):
                    nc.vector.scalar_tensor_tensor(
                        ss_g[:, m, :],
                        ss_g[:, m - 1, :],
                        bdm_sb[:, g, :],
                        c_sb[:, g, m - 1, :],
                        ALU.mult,
                        ALU.add,
                    )
                pin = ps2.tile([128, NBLK, BS], F32, name="pin", bufs=2)
                for m in range(NBLK):
                    for hh in range(2):
                        hs = slice(hh * 64, (hh + 1) * 64)
                        nc.tensor.matmul(
                            pin[hs, m, :],
                            ss_g[hs, m, :],
                            qT_sb[hs, g, m * BS : (m + 1) * BS],
                            start=True,
                            stop=True,
                            skip_group_check=True,
                        )
                tmp3 = tmp_p.tile([128, NBLK, BS], BF16, name="tmp3")
                nc.vector.tensor_mul(
                    tmp3[:],
                    pin[:],
                    qd_sb[:, g : g + 1, :].broadcast_to([128, NBLK, BS]),
                )
                nc.vector.tensor_add(
                    ys_sb[:, g, :],
                    ys_sb[:, g, :],
                    tmp3[:].rearrange("p a b -> p (a b)"),
                )
                # rmsnorm prep: accumulate sum over groups of ys^2 in SBUF
                sq = sq_p2.tile([128, T], F32, name="sq")
                nc.scalar.activation(sq[:], ys_sb[:, g, :], AF.Square)
                if first:
                    nc.vector.tensor_copy(acc[:], sq[:])
                else:
                    nc.vector.tensor_add(acc[:], acc[:], sq[:])

            def pass2_partB(g):
                # gate <- ys * norm_w * gate; needs group g's sigmoid.
                # With phase F consuming the last gate chunks last, this
                # never gates the PE.
                nc.vector.scalar_tensor_tensor(
                    gate_sb[:, g, :],
                    ys_sb[:, g, :],
                    nw_sb[:, g : g + 1],
                    gate_sb[:, g, :],
                    ALU.mult,
                    ALU.mult,
                )

            # partA runs one gate chunk ahead of partB; by the last gate
            # chunk every rmsnorm contribution is in, so the finalize
            # (emitted after partA of the last group) hides under it and
            # only two cheap gate-STTs trail the final gate matmuls
            g_order = list(range(GSPLIT, G)) + list(range(GSPLIT))
            for i in range(G):
                gm = g_order[i]
                gw_t = gw_p.tile([128, KC, 128], BF16, name="gw_t")
                nc.sync.dma_start(
                    gw_t[:],
                    gwT[:, gm * 128 : (gm + 1) * 128].rearrange(
                        "(ko p) c -> p ko c", p=128
                    ),
                )
                for gn in range(2):
                    pg = ps_g.tile([128, 512], F32, name="pg", bufs=2)
                    for k in range(KC):
                        nc.tensor.matmul(
                            pg[:],
                            gw_t[:, k, :],
                            xT[:, k, gn * 512 : (gn + 1) * 512],
                            start=(k == 0),
                            stop=(k == KC - 1),
                        )
                    nc.scalar.activation(
                        gate_sb[:, gm, gn * 512 : (gn + 1) * 512],
                        pg[:],
                        AF.Sigmoid,
                    )
                if i == 5:
                    eg_accum(0)
                if i >= 1:
                    pass2_partB(g_order[i - 1])
                if i == 0:
                    pass2_partA(g_order[0], first=True)
                    pass2_partA(g_order[1])
                elif i + 1 < G:
                    pass2_partA(g_order[i + 1])
            pass2_partB(g_order[G - 1])

            # rmsnorm scale per token: reduce acc over partitions
            acc_bf = sq_p2.tile([128, NCH, 128], BF16, name="acc_bf")
            nc.scalar.copy(
                acc_bf[:].rearrange("p a b -> p (a b)"), acc[:]
            )
            if USE_PNS:
                # 8 tiny matmuls (stationary = token chunk, moving = ones)
                # -> [128 tok, 1] per chunk, the layout phase F needs
                pns = ps_sq.tile([128, NCH], F32, name="pns")
                for m in range(NCH):
                    nc.tensor.matmul(
                        pns[:, m : m + 1],
                        acc_bf[:, m, :],
                        ones[:],
                        start=True,
                        stop=True,
                        skip_group_check=True,
                    )
                nc.scalar.activation(
                    ns_t[:], pns[:], AF.Sqrt, bias=eps_sb[:, 0:1], scale=1.0 / (H * D)
                )
                nc.vector.reciprocal(ns_sb[:], ns_t[:])
            else:
                # baseline-style: ones^T @ acc -> [1, T], DRAM roundtrip to
                # redistribute tokens across partitions
                ssq = ps_sq.tile([1, T], F32, name="ssq")
                for half in range(2):
                    nc.tensor.matmul(
                        ssq[:, half * 512 : (half + 1) * 512],
                        ones[:],
                        acc_bf[:].rearrange("p a b -> p (a b)")[
                            :, half * 512 : (half + 1) * 512
                        ],
                        start=True,
                        stop=True,
                        skip_group_check=True,
                    )
                ssq_sb = sq_p2.tile([1, T], F32, name="ssq_sb")
                nc.vector.tensor_copy(ssq_sb[:], ssq[:])
                nc.sync.dma_start(ssq_rt[:], ssq_sb[:])
                ns_l = sq_p2.tile([128, NCH], F32, name="ns_l")
                nc.sync.dma_start(ns_l[:], ssq_rt.rearrange("(c p) -> p c", p=128))
                nc.scalar.activation(
                    ns_t[:], ns_l[:], AF.Sqrt, bias=eps_sb[:, 0:1], scale=1.0 / (H * D)
                )
                nc.vector.reciprocal(ns_sb[:], ns_t[:])

        # ---- phase F: output projection --------------------------------
        with tc.tile_pool(
            name="oo_p", bufs=1
        ) as oo_p, tc.tile_pool(name="ps_o", bufs=1, space="PSUM") as ps_o:
            # consume gate chunks in the order pass 2 finishes them
            # (12..15 first, 11 last) so the first n-pass never waits on
            # the final pass2 group's vector tail
            k2_order = [6, 7, 0, 1, 2, 3, 4, 5]
            for n in range(4):
                po = [
                    ps_o.tile([128, 512], F32, name=f"po{m}") for m in range(NCH)
                ]
                for ki, k2 in enumerate(k2_order):
                    ow_t = ow_p.tile([128, 2, 512], BF16, name="ow_t")
                    # gpsimd queue: idle by now, so ow never waits behind
                    # the gw triggers' buffer-free stalls on sync
                    nc.gpsimd.dma_start(
                        ow_t[:],
                        owT[
                            k2 * 256 : (k2 + 1) * 256, n * 512 : (n + 1) * 512
                        ].rearrange("(ko p) c -> p ko c", p=128),
                    )
                    for kk in range(2):
                        k = 2 * k2 + kk
                        for m in range(NCH):
                            nc.tensor.matmul(
                                po[m][:],
                                gate_sb[:, k, m * 128 : (m + 1) * 128],
                                ow_t[:, kk, :],
                                start=(ki == 0 and kk == 0),
                                stop=(ki == len(k2_order) - 1 and kk == 1),
                            )
                oo_all = oo_p.tile([128, NCH, 512], F32, name="oo_all")
                for mh in range(4):
                    for mm in range(2):
                        m = 2 * mh + mm
                        nc.vector.tensor_scalar_mul(
                            oo_all[:, m, :], po[m][:], ns_sb[:, m : m + 1]
                        )
                    # out writes alternate across idle queues so the final
                    # writes never back up behind one another
                    if USE_ACT_QUEUE:
                        oq = nc.scalar if mh % 2 == 0 else nc.sync
                    else:
                        oq = nc.sync if mh % 2 == 0 else nc.gpsimd
                    oq.dma_start(
                        out[
                            mh * 256 : (mh + 1) * 256,
                            n * 512 : (n + 1) * 512,
                        ].rearrange("(m p) c -> p m c", p=128),
                        oo_all[:, mh * 2 : (mh + 1) * 2, :],
                    )
        _g_ctx.close()
        _ow_ctx.close()
        _gw_ctx.close()
        _ce_ctx.close()
        _ys_ctx.close()
        _q_ctx.close()
        _c_ctx.close()
        _xt_ctx.close()
    nc.finalize()
    return nc


_CACHE = {}


def _get_nc():
    if "nc" not in _CACHE:
        _CACHE["nc"] = _build_nc()
    return _CACHE["nc"]


def _host_prep(hidden_states, qkv_w, out_w, gate_w, norm_w):
    slope, qd, kd, dd, bd = _decay()
    w3 = qkv_w.reshape(H, 3 * D, HID)
    wq = w3[:, 0:D, :].reshape(H * D, HID)
    wk = w3[:, D : 2 * D, :].reshape(H * D, HID)
    wv = w3[:, 2 * D : 3 * D, :].reshape(H * D, HID)
    # wqkT[:, g, 0:128] = q dims of heads 2g,2g+1; [:, g, 128:256] = k dims
    wqkT = np.concatenate(
        [
            np.ascontiguousarray(wq.T).reshape(HID, G, 128),
            np.ascontiguousarray(wk.T).reshape(HID, G, 128),
        ],
        axis=2,
    ).astype(BF)
    wvT = np.ascontiguousarray(wv.T).astype(BF)
    gwT = np.ascontiguousarray(gate_w.T).astype(BF)
    owT = np.ascontiguousarray(out_w.T).astype(BF)

    # ddm[g, p, hh*128 + i] = dd[2g+hh, i, p]   (p = key token j)
    dd_t = dd.transpose(0, 2, 1)  # (H, j, i)
    ddm = np.ascontiguousarray(
        dd_t.reshape(G, 2, 128, BS).transpose(0, 2, 1, 3).reshape(G, 128, 256)
    ).astype(BF)
    # qdm[p, g, i] = qd[2g + p//64, i]
    qdm = np.ascontiguousarray(
        np.broadcast_to(qd.reshape(G, 2, 1, BS), (G, 2, 64, BS))
        .transpose(1, 2, 0, 3)
        .reshape(128, G, BS)
    ).astype(BF)
    # kdm[p, g, hh*64+d] = kd[2g+hh, p]
    kdm = np.ascontiguousarray(
        np.broadcast_to(
            kd.T.reshape(128, G, 2, 1), (128, G, 2, 64)
        ).reshape(128, G, 128)
    ).astype(BF)
    # bdm[p, g] = bd[2g + p//64]
    bdm = np.ascontiguousarray(
        np.broadcast_to(bd.reshape(G, 2, 1), (G, 2, 64)).transpose(1, 2, 0).reshape(128, G)
    ).astype(np.float32)
    nw = np.ascontiguousarray(norm_w.reshape(16, 128).T).astype(np.float32)

    shared = dict(wqkT=wqkT, wvT=wvT, gwT=gwT, owT=owT, ddm=ddm, qdm=qdm,
                  kdm=kdm, bdm=bdm, nw=nw)
    in_maps = []
    for c in range(NC):
        bb, p = c // 4, c % 4
        hsT = np.ascontiguousarray(
            hidden_states[bb, p * T : (p + 1) * T, :].T
        ).astype(BF)
        sw = np.zeros((H, NC), dtype=np.float64)
        for cc in range(NC):
            if cc // 4 == bb and cc % 4 < p:
                sw[:, cc] = bd ** (NBLK * (p - 1 - (cc % 4)))
        # swm[p_, cc*G+g] = sw[2g + p_//64, cc]  (cc-major)
        swm = np.ascontiguousarray(
            np.broadcast_to(sw.reshape(G, 2, 1, NC), (G, 2, 64, NC))
            .transpose(1, 2, 3, 0)
            .reshape(128, NC * G)
        ).astype(np.float32)
        in_maps.append(dict(hsT=hsT, swm=swm, **shared))
    return in_maps


def _run(inputs, trace=False):
    nc = _get_nc()
    in_maps = _host_prep(
        np.asarray(inputs["hidden_states"], dtype=np.float32),
        np.asarray(inputs["qkv_w"], dtype=np.float32),
        np.asarray(inputs["out_w"], dtype=np.float32),
        np.asarray(inputs["gate_w"], dtype=np.float32),
        np.asarray(inputs["norm_w"], dtype=np.float32),
    )
    res = run_bass_kernel_spmd(nc, in_maps, core_ids=list(range(NC)), trace=trace)
    full = np.empty((B, S, HID), dtype=np.float32)
    for c in range(NC):
        bb, p = c // 4, c % 4
        full[bb, p * T : (p + 1) * T, :] = res.results[c]["out"]
    return full, res


def kernel(**inputs):
    return _run(inputs, trace=False)[0]


def kernel_traced(**inputs):
    full, res = _run(inputs, trace=True)
    return full, res.exec_time_ns
